# revision 1
# baseline (speedup 1.0000x reference)
"""Trainium2 Bass kernel for CoRA/AdaLoRA embedding lookup.

Computes: out = (E + scaling * lora_B @ (lora_A * mask))[x]  for
  E [500000, 128] f32, lora_B [500000, 8] f32, lora_A [8, 128] f32,
  rank_pattern [8] f32, x [4096, 200] int.

Strategy: pure data-parallel over the batch across 8 NeuronCores with the
table replicated.  Per core, tokens are bucketed by vocab bank (16 banks of
31250 rows, so in-bank indices fit int16) on the host.  Rows of a fused
table [E | lora_B | pad] (768 B, dma_gather needs elem%256B==0) are gathered
with gpsimd.dma_gather in chunks of 1024 indices (ucode descriptor-ring
limit) with -1 tail padding (skipped by HW).  The rank-8 LoRA delta is
computed on-chip (PE transpose + block-diagonal K=64 matmul) and added in
place to the gathered embedding columns, which are DMAd out per bank.  The
host un-permutes the sorted output.  Per-core HBM traffic ~140 MB.
"""

import numpy as np

V = 500000
D = 128
R = 8
EROW = 192             # fused row: 128 emb + 8 lora_B + 56 pad = 768 B
SCALING = 2.0          # LORA_ALPHA / R = 16 / 8
THRESH = 0.1
B, L = 4096, 200
NCORES = 8
P = 128
TPC = B * L // NCORES  # 102400 tokens per core

NBANK = 16
BW = V // NBANK        # 31250 (< 2^15, in-bank index fits int16)
NSUB = 7               # chunks (= compute subtiles) per bank
G = 8                  # dst columns (of 128 slots) per chunk
CHUNK = G * P          # 1024 idxs per dma_gather (HW ring limit)
CAP = NSUB * CHUNK     # 7168 slots per bank (static capacity)
CCOL = CAP // P        # 56 dst columns per bank
ICOL = CAP // 16       # 448 idx columns per bank
NCOL = NBANK * CCOL    # 896 total out columns


def build_nc(nbank=NBANK, bw=BW, nsub=NSUB):
    from concourse import bass, bacc, mybir
    from concourse.library_config import mlp
    from contextlib import ExitStack

    f32 = mybir.dt.float32
    bf16 = mybir.dt.bfloat16
    i16 = mybir.dt.int16
    cap = nsub * CHUNK
    ccol = cap // P
    icol = cap // 16
    ichk = CHUNK // 16  # 64 idx columns per chunk
    v = nbank * bw
    nsubt = nbank * nsub  # total subtiles == total gathers

    nc = bacc.Bacc(num_swdge_queues=3)
    tab = nc.declare_dram_parameter("tab", [v, EROW], f32, False)
    idx = nc.declare_dram_parameter("idx", [P, nbank * icol], i16, False)
    cnts = nc.declare_dram_parameter("cnts", [1, nbank * nsub], mybir.dt.int32, False)
    aeffb = nc.declare_dram_parameter("aeffb", [G * R, G * D], bf16, False)
    ident = nc.declare_dram_parameter("ident", [P, P], f32, False)
    out = nc.declare_dram_parameter("out", [P, nbank * ccol, D], f32, True)

    with ExitStack() as st:
        block = st.enter_context(nc.Block())
        idx_sb = st.enter_context(nc.sbuf_tensor("idx_sb", [P, nbank * icol], i16))
        cnts_sb = st.enter_context(
            nc.sbuf_tensor("cnts_sb", [1, nbank * nsub], mybir.dt.int32)
        )
        aug = [
            st.enter_context(nc.sbuf_tensor(f"aug{i}", [P, ccol, EROW], f32))
            for i in range(3)
        ]
        lb_cont = [
            st.enter_context(nc.sbuf_tensor(f"lbc{i}", [P, G * R], f32))
            for i in range(2)
        ]
        lbT = [
            st.enter_context(nc.sbuf_tensor(f"lbT{i}", [G * R, P], bf16))
            for i in range(2)
        ]
        ident_sb = st.enter_context(nc.sbuf_tensor("ident_sb", [P, P], f32))
        aeff_sb = st.enter_context(nc.sbuf_tensor("aeff_sb", [G * R, G * D], bf16))
        pt_full = [
            st.enter_context(nc.psum_tensor(f"pt{i}", [G * R, 512], f32))
            for i in range(2)
        ]
        pm = [
            [
                st.enter_context(nc.psum_tensor(f"pm{i}_{q}", [P, 512], f32))
                for q in range(2)
            ]
            for i in range(2)
        ]
        io_sem = st.enter_context(nc.semaphore("io_sem"))
        ix_sem = st.enter_context(nc.semaphore("ix_sem"))
        z_sem = st.enter_context(nc.semaphore("z_sem"))
        g_sems = [st.enter_context(nc.semaphore(f"g_sem{i}")) for i in range(3)]
        o_sem = st.enter_context(nc.semaphore("o_sem"))
        d1_sem = st.enter_context(nc.semaphore("d1_sem"))
        d2_sem = st.enter_context(nc.semaphore("d2_sem"))
        d3_sem = st.enter_context(nc.semaphore("d3_sem"))
        pe_sem = st.enter_context(nc.semaphore("pe_sem"))

        @block.gpsimd
        def _(gp: "bass.BassGpSimd"):
            gp.load_library(mlp)
            gp.wait_ge(ix_sem, 32)  # idx + counts loaded
            with gp.register("cnt") as cnt_reg:
                for b in range(nbank):
                    pe_ = b % 3
                    if b < 3:
                        gp.wait_ge(z_sem, pe_ + 1)  # aug[pe_] zeroed
                    else:
                        gp.wait_ge(o_sem, 32 * (b - 2))  # out DMAs of bank b-3 done
                    for s in range(nsub):
                        k = b * nsub + s
                        gp.reg_load(cnt_reg, cnts_sb[0:1, k : k + 1])
                        cnt = gp.snap(cnt_reg)
                        gp.dma_gather(
                            aug[pe_][:, s * G : (s + 1) * G, :],
                            tab[b * bw : (b + 1) * bw, :],
                            idx_sb[:, b * icol + s * ichk : b * icol + (s + 1) * ichk],
                            CHUNK,
                            cnt,
                            EROW,
                            queue_num=pe_,
                        ).then_inc(g_sems[pe_], 16)

        @block.vector
        def _(ve: "bass.BassVectorEngine"):
            for i in range(3):
                ve.memset(aug[i][:, :, :], 0.0).then_inc(z_sem, 1)
            # prologue: lb_cont for subtile 0 (whole bank 0 gathered)
            ve.wait_ge(g_sems[0], 16 * nsub)
            ve.tensor_copy(
                out=lb_cont[0][:, :], in_=aug[0][:, 0:G, D : D + R]
            ).then_inc(d1_sem, 1)
            for n in range(nsubt):
                b, s = divmod(n, nsub)
                pe_ = b % 3
                # lbT copy (needs PE transpose n)
                ve.wait_ge(pe_sem, 2 * n + 1)
                ve.tensor_copy(
                    out=lbT[n % 2][:, :], in_=pt_full[n % 2][:, 0:P]
                ).then_inc(d2_sem, 1)
                # software-pipelined lb_cont for subtile n+1
                if n + 1 < nsubt:
                    b2, s2 = divmod(n + 1, nsub)
                    if s2 == 0:
                        ve.wait_ge(g_sems[b2 % 3], 16 * nsub * (b2 // 3 + 1))
                    ve.tensor_copy(
                        out=lb_cont[(n + 1) % 2][:, :],
                        in_=aug[b2 % 3][:, s2 * G : (s2 + 1) * G, D : D + R],
                    ).then_inc(d1_sem, 1)
                # adds (need PE matmuls n); in-place into the emb columns
                ve.wait_ge(pe_sem, 2 * n + 2)
                half = G // 2
                ve.tensor_add(
                    out=aug[pe_][:, s * G : s * G + half, 0:D],
                    in0=aug[pe_][:, s * G : s * G + half, 0:D],
                    in1=pm[n % 2][0][:, :],
                )
                ve.tensor_add(
                    out=aug[pe_][:, s * G + half : (s + 1) * G, 0:D],
                    in0=aug[pe_][:, s * G + half : (s + 1) * G, 0:D],
                    in1=pm[n % 2][1][:, :],
                ).then_inc(d3_sem, 1)

        @block.tensor
        def _(te: "bass.BassTensorEngine"):
            te.wait_ge(io_sem, 32)  # ident + aeff loaded
            for n in range(nsubt):
                te.wait_ge(d1_sem, n + 1)
                if n >= 2:
                    te.wait_ge(d2_sem, n - 1)  # WAR pt[n%2]
                te.transpose(
                    out=pt_full[n % 2][:, 0:P],
                    in_=lb_cont[n % 2][:, :],
                    identity=ident_sb[:, :],
                ).then_inc(pe_sem, 1)
                te.wait_ge(d2_sem, n + 1)      # lbT ready
                if n >= 2:
                    te.wait_ge(d3_sem, n - 1)  # WAR pm[n%2]
                te.matmul(
                    out=pm[n % 2][0][:, :],
                    lhsT=lbT[n % 2][:, :],
                    rhs=aeff_sb[:, 0:512],
                    start=True,
                    stop=True,
                )
                te.matmul(
                    out=pm[n % 2][1][:, :],
                    lhsT=lbT[n % 2][:, :],
                    rhs=aeff_sb[:, 512:1024],
                    start=True,
                    stop=True,
                ).then_inc(pe_sem, 1)

        @block.sync
        def _(sy: "bass.BassEngine"):
            sy.dma_start(out=idx_sb[:, :], in_=idx[:, :]).then_inc(ix_sem, 16)
            sy.dma_start(out=cnts_sb[:, :], in_=cnts[:, :]).then_inc(ix_sem, 16)
            sy.dma_start(out=ident_sb[:, :], in_=ident[:, :]).then_inc(io_sem, 16)
            sy.dma_start(out=aeff_sb[:, :], in_=aeffb[:, :]).then_inc(io_sem, 16)
            hc = (nsub // 2 + 1) * G  # 32 cols after subtiles 0..3
            for b in range(nbank):
                sy.wait_ge(d3_sem, nsub * b + nsub // 2 + 1)
                sy.dma_start(
                    out=out[:, b * ccol : b * ccol + hc, :],
                    in_=aug[b % 3][:, 0:hc, 0:D],
                ).then_inc(o_sem, 16)
                sy.wait_ge(d3_sem, nsub * (b + 1))
                sy.dma_start(
                    out=out[:, b * ccol + hc : (b + 1) * ccol, :],
                    in_=aug[b % 3][:, hc:ccol, 0:D],
                ).then_inc(o_sem, 16)
            sy.wait_ge(o_sem, 32 * nbank)

    nc.compile()
    return nc


_NC_CACHE = {}


def _get_nc():
    if "nc" not in _NC_CACHE:
        _NC_CACHE["nc"] = build_nc()
    return _NC_CACHE["nc"]


def _wrap16(lst):
    """Token t -> (t % 16, t // 16), tiled 8x across 128 partitions."""
    blk = lst.reshape(-1, 16).T  # [16, n/16]
    return np.tile(blk, (8, 1))


def prepare_in_maps(x, embedding_weight, lora_A, lora_B, rank_pattern):
    x = np.asarray(x)
    E = np.asarray(embedding_weight, dtype=np.float32)
    A = np.asarray(lora_A, dtype=np.float32)
    LB = np.asarray(lora_B, dtype=np.float32)
    rp = np.asarray(rank_pattern, dtype=np.float32)

    import ml_dtypes

    a_scaled = A * (rp > THRESH).astype(np.float32)[:, None] * np.float32(SCALING)
    aeffb = np.zeros((G * R, G * D), dtype=ml_dtypes.bfloat16)
    for gg in range(G):
        aeffb[gg * R : (gg + 1) * R, gg * D : (gg + 1) * D] = a_scaled
    tab = np.zeros((V, EROW), dtype=np.float32)
    tab[:, :D] = E
    tab[:, D : D + R] = LB
    ident = np.eye(P, dtype=np.float32)

    xi = x.reshape(-1).astype(np.int64)
    in_maps = []
    host_info = []
    for c in range(NCORES):
        xc = xi[c * TPC : (c + 1) * TPC]
        bank = xc // BW
        within = (xc - bank * BW).astype(np.int16)
        order = np.argsort(bank, kind="stable")
        counts = np.bincount(bank, minlength=NBANK).astype(np.int64)
        overflow = {}
        idx16 = np.full((P, NBANK * ICOL), -1, dtype=np.int16)
        takes = np.zeros(NBANK * NSUB, dtype=np.int32)
        start = 0
        for b in range(NBANK):
            nb = int(counts[b])
            take = min(nb, CAP)
            lst = np.full(CAP, -1, dtype=np.int16)
            lst[:take] = within[order[start : start + take]]
            if nb > CAP:  # pathological: handle the excess on the host
                overflow[b] = order[start + take : start + nb]
            # per-chunk valid counts (chunks are filled front to back)
            for s in range(NSUB):
                t = min(max(take - s * CHUNK, 0), CHUNK)
                if t == 0:  # ucode needs >=1 valid index; slot is discarded
                    lst[s * CHUNK] = 0
                    t = 1
                takes[b * NSUB + s] = t
            idx16[:, b * ICOL : (b + 1) * ICOL] = _wrap16(lst)
            start += nb
        in_maps.append(
            {
                "tab": tab,
                "idx": idx16,
                "cnts": takes.reshape(1, NBANK * NSUB),
                "aeffb": aeffb,
                "ident": ident,
            }
        )
        host_info.append((order, counts, overflow))
    return in_maps, host_info, (E, LB, a_scaled)


def collect(results, host_info, tabs, x):
    """Un-sort the banked output; host-patches (never-in-practice) bank overflow."""
    E, LB, a_scaled = tabs
    xi = np.asarray(x).reshape(-1).astype(np.int64)
    cores = []
    for c in range(NCORES):
        order, counts, overflow = host_info[c]
        oc = np.asarray(results[c]["out"])
        flat = oc.transpose(1, 0, 2).reshape(NCOL * P, D)
        core_out = np.empty((TPC, D), dtype=np.float32)
        src_slots = np.concatenate(
            [np.arange(min(int(counts[b]), CAP)) + b * CAP for b in range(NBANK)]
        )
        starts = np.concatenate([[0], np.cumsum(counts)]).astype(np.int64)
        dst_tok = np.concatenate(
            [order[starts[b] : starts[b] + min(int(counts[b]), CAP)] for b in range(NBANK)]
        )
        core_out[dst_tok] = flat[src_slots]
        for b, toks in overflow.items():
            ids = xi[c * TPC + toks]
            core_out[toks] = E[ids] + LB[ids] @ a_scaled
        cores.append(core_out)
    return np.concatenate(cores, axis=0).reshape(B, L, D)


def kernel(x, embedding_weight, lora_A, lora_B, rank_pattern):
    from concourse.bass_utils import run_bass_kernel_spmd

    x = np.asarray(x)
    in_maps, host_info, tabs = prepare_in_maps(
        x, embedding_weight, lora_A, lora_B, rank_pattern
    )
    nc = _get_nc()
    res = run_bass_kernel_spmd(nc, in_maps, list(range(NCORES))).results
    return collect(res, host_info, tabs, x)



# revision 2
# speedup vs baseline: 1.5256x; 1.5256x over previous
"""Trainium2 Bass kernel for CoRA/AdaLoRA embedding lookup.

Computes: out = (E + scaling * lora_B @ (lora_A * mask))[x]  for
  E [500000, 128] f32, lora_B [500000, 8] f32, lora_A [8, 128] f32,
  rank_pattern [8] f32, x [4096, 200] int.

Strategy: data-parallel over tokens across 8 NeuronCores, with per-bank
round-robin core assignment so every (core, vocab-bank) bucket is balanced
(~6400 +- 30 tokens).  The table is stored bf16 (rel tolerance 2e-2): each
gathered row is exactly 256 B, the dma_gather minimum, so zero pad traffic
(vs 768 B fused-f32 rows before).  lora_B[x] is gathered on the host and
streamed in pre-transposed ([64, chunks*128] bf16), so the on-chip pipeline
is just: gpsimd dma_gather (4 SWDGE queues = 4 rotating bank buffers) ->
PE block-diagonal K=64 matmul for the rank-8 delta (gated only by PSUM
reuse, never by gathers) -> DVE in-place add (bf16) -> HWDGE store of bf16
rows.  All gather chunks use constant counts (padding indices point at row
0) so no per-chunk register loads are needed.  The host un-permutes and
upcasts to f32.  Per-core HBM traffic ~57 MB (was ~139 MB).
"""

import numpy as np

V = 500000
D = 128
R = 8
SCALING = 2.0          # LORA_ALPHA / R = 16 / 8
THRESH = 0.1
B, L = 4096, 200
NCORES = 8
P = 128
NTOK = B * L           # 819200 tokens total

NBANK = 16
BW = V // NBANK        # 31250 (< 2^15, in-bank index fits int16)
NSUB = 7               # gathers per bank: 6 x 1024 + 1 x 512
CHUNK_N = [1024] * 6 + [512]
CAP = sum(CHUNK_N)     # 6656 slots per (core, bank)
CCOL = CAP // P        # 52 dst columns per bank
NCOL = NBANK * CCOL    # 832 total out columns
ICOLB = CAP // 16      # 416 idx columns per bank
NCHUNK = NBANK * NSUB  # 112 chunks per core
COFF = [0, 8, 16, 24, 32, 40, 48]        # dst col offset per sub
IOFF = [0, 64, 128, 192, 256, 320, 384]  # idx col offset per sub
NQ = 4                 # SWDGE queues == rotating bank buffers


def build_nc():
    from concourse import bass, bacc, mybir
    from concourse.library_config import mlp
    from contextlib import ExitStack

    f32 = mybir.dt.float32
    bf16 = mybir.dt.bfloat16
    i16 = mybir.dt.int16

    nc = bacc.Bacc(num_swdge_queues=NQ)
    tab = nc.declare_dram_parameter("tab", [V, D], bf16, False)
    idx = nc.declare_dram_parameter("idx", [P, NBANK * ICOLB], i16, False)
    lbt = nc.declare_dram_parameter("lbt", [8 * R, NCHUNK * P], bf16, False)
    aeffb = nc.declare_dram_parameter("aeffb", [8 * R, 8 * D], bf16, False)
    out = nc.declare_dram_parameter("out", [P, NCOL, D], bf16, True)

    with ExitStack() as st:
        block = st.enter_context(nc.Block())
        idx_sb = st.enter_context(nc.sbuf_tensor("idx_sb", [P, NBANK * ICOLB], i16))
        lbt_sb = st.enter_context(nc.sbuf_tensor("lbt_sb", [8 * R, NCHUNK * P], bf16))
        aeff_sb = st.enter_context(nc.sbuf_tensor("aeff_sb", [8 * R, 8 * D], bf16))
        aug = [
            st.enter_context(nc.sbuf_tensor(f"aug{i}", [P, CCOL, D], bf16))
            for i in range(NQ)
        ]
        pm = [
            [
                st.enter_context(nc.psum_tensor(f"pm{i}_{j}", [P, 512], f32))
                for j in range(2)
            ]
            for i in range(2)
        ]
        ix_sem = st.enter_context(nc.semaphore("ix_sem"))
        g_sems = [st.enter_context(nc.semaphore(f"g_sem{i}")) for i in range(NQ)]
        pe_sem = st.enter_context(nc.semaphore("pe_sem"))
        d3_sem = st.enter_context(nc.semaphore("d3_sem"))
        o_sem = st.enter_context(nc.semaphore("o_sem"))

        @block.gpsimd
        def _(gp: "bass.BassGpSimd"):
            gp.load_library(mlp)
            gp.wait_ge(ix_sem, 16)  # idx loaded
            for b in range(NBANK):
                q = b % NQ
                if b >= NQ:
                    gp.wait_ge(o_sem, 32 * (b - NQ + 1))  # bank b-NQ stored
                for s in range(NSUB):
                    ni = CHUNK_N[s]
                    gp.dma_gather(
                        aug[q][:, COFF[s] : COFF[s] + ni // P, :],
                        tab[b * BW : (b + 1) * BW, :],
                        idx_sb[:, b * ICOLB + IOFF[s] : b * ICOLB + IOFF[s] + ni // 16],
                        ni,
                        ni,
                        D,
                        queue_num=q,
                    ).then_inc(g_sems[q], 16)

        @block.tensor
        def _(te: "bass.BassTensorEngine"):
            te.wait_ge(ix_sem, 48)  # lbt + aeff loaded
            for n in range(NCHUNK):
                s = n % NSUB
                if n >= 2:
                    te.wait_ge(d3_sem, n - 1)  # WAR pm[n%2]
                lb = lbt_sb[:, n * P : (n + 1) * P]
                if s < 6:
                    te.matmul(
                        out=pm[n % 2][0][:, :],
                        lhsT=lb,
                        rhs=aeff_sb[:, 0:512],
                        start=True,
                        stop=True,
                    )
                    te.matmul(
                        out=pm[n % 2][1][:, :],
                        lhsT=lb,
                        rhs=aeff_sb[:, 512:1024],
                        start=True,
                        stop=True,
                    ).then_inc(pe_sem, 1)
                else:
                    te.matmul(
                        out=pm[n % 2][0][:, :],
                        lhsT=lb,
                        rhs=aeff_sb[:, 0:512],
                        start=True,
                        stop=True,
                    ).then_inc(pe_sem, 1)

        @block.vector
        def _(ve: "bass.BassVectorEngine"):
            for n in range(NCHUNK):
                b, s = divmod(n, NSUB)
                q = b % NQ
                r = b // NQ
                ve.wait_ge(g_sems[q], 16 * (NSUB * r + s + 1))
                ve.wait_ge(pe_sem, n + 1)
                c0 = COFF[s]
                if s < 6:
                    ve.tensor_add(
                        out=aug[q][:, c0 : c0 + 4, :],
                        in0=aug[q][:, c0 : c0 + 4, :],
                        in1=pm[n % 2][0][:, :],
                    )
                    ve.tensor_add(
                        out=aug[q][:, c0 + 4 : c0 + 8, :],
                        in0=aug[q][:, c0 + 4 : c0 + 8, :],
                        in1=pm[n % 2][1][:, :],
                    ).then_inc(d3_sem, 1)
                else:
                    ve.tensor_add(
                        out=aug[q][:, c0 : c0 + 4, :],
                        in0=aug[q][:, c0 : c0 + 4, :],
                        in1=pm[n % 2][0][:, :],
                    ).then_inc(d3_sem, 1)

        @block.sync
        def _(sy: "bass.BassEngine"):
            sy.dma_start(out=idx_sb[:, :], in_=idx[:, :]).then_inc(ix_sem, 16)
            sy.dma_start(out=lbt_sb[:, :], in_=lbt[:, :]).then_inc(ix_sem, 16)
            sy.dma_start(out=aeff_sb[:, :], in_=aeffb[:, :]).then_inc(ix_sem, 16)
            for b in range(NBANK):
                q = b % NQ
                sy.wait_ge(d3_sem, NSUB * b + 4)
                sy.dma_start(
                    out=out[:, b * CCOL : b * CCOL + 32, :],
                    in_=aug[q][:, 0:32, :],
                ).then_inc(o_sem, 16)
                sy.wait_ge(d3_sem, NSUB * (b + 1))
                sy.dma_start(
                    out=out[:, b * CCOL + 32 : (b + 1) * CCOL, :],
                    in_=aug[q][:, 32:CCOL, :],
                ).then_inc(o_sem, 16)
            sy.wait_ge(o_sem, 32 * NBANK)

    nc.compile()
    return nc


_NC_CACHE = {}


def _get_nc():
    if "nc" not in _NC_CACHE:
        _NC_CACHE["nc"] = build_nc()
    return _NC_CACHE["nc"]


def _wrap16(lst):
    """Token t -> (t % 16, t // 16), tiled 8x across 128 partitions."""
    blk = lst.reshape(-1, 16).T  # [16, n/16]
    return np.tile(blk, (8, 1))


# static slot -> (partition, in-bank column) maps
_J = np.arange(CAP)
_PMAP = np.where(_J < 6144, (_J % 1024) % P, (_J - 6144) % P).astype(np.int64)
_CMAP = np.where(
    _J < 6144, (_J // 1024) * 8 + (_J % 1024) // P, 48 + (_J - 6144) // P
).astype(np.int64)


def prepare_in_maps(x, embedding_weight, lora_A, lora_B, rank_pattern):
    import ml_dtypes

    x = np.asarray(x)
    E = np.asarray(embedding_weight, dtype=np.float32)
    A = np.asarray(lora_A, dtype=np.float32)
    LB = np.asarray(lora_B, dtype=np.float32)
    rp = np.asarray(rank_pattern, dtype=np.float32)

    a_scaled = A * (rp > THRESH).astype(np.float32)[:, None] * np.float32(SCALING)
    aeffb = np.zeros((8 * R, 8 * D), dtype=ml_dtypes.bfloat16)
    for g in range(8):
        aeffb[g * R : (g + 1) * R, g * D : (g + 1) * D] = a_scaled
    tab = E.astype(ml_dtypes.bfloat16)
    LBb = LB.astype(ml_dtypes.bfloat16)

    xi = x.reshape(-1).astype(np.int64)
    bank = xi // BW
    order = np.argsort(bank, kind="stable")
    counts_g = np.bincount(bank, minlength=NBANK)
    starts_g = np.concatenate([[0], np.cumsum(counts_g)]).astype(np.int64)

    in_maps = []
    host_info = []
    for c in range(NCORES):
        parts = []          # per bank: this core's token positions (clipped)
        n_clip = np.zeros(NBANK, dtype=np.int64)
        overflow = {}
        for b in range(NBANK):
            lst = order[starts_g[b] : starts_g[b + 1]][c::NCORES]
            if len(lst) > CAP:  # pathological; host patches the excess
                overflow[b] = lst[CAP:]
                lst = lst[:CAP]
            n_clip[b] = len(lst)
            parts.append(lst)
        tokens_c = np.concatenate(parts)
        valid = np.arange(CAP)[None, :] < n_clip[:, None]   # [16, CAP]

        within = np.zeros((NBANK, CAP), dtype=np.int16)
        within[valid] = (xi[tokens_c] - bank[tokens_c] * BW).astype(np.int16)
        ids_pad = np.zeros((NBANK, CAP), dtype=np.int64)
        ids_pad[valid] = xi[tokens_c]
        slot_src = np.full((NBANK, CAP), -1, dtype=np.int64)
        slot_src[valid] = tokens_c

        idx16 = np.empty((P, NBANK * ICOLB), dtype=np.int16)
        for b in range(NBANK):
            idx16[:, b * ICOLB : (b + 1) * ICOLB] = _wrap16(within[b])

        # pre-transposed lora_B[x]: lbt[g*8+r, n*128+p] = LB[id(slot n,g,p), r]
        LBv = LBb[ids_pad.reshape(-1)]  # [16*CAP, 8] bf16
        lbt = np.zeros((8 * R, NCHUNK * P), dtype=ml_dtypes.bfloat16)
        for b in range(NBANK):
            Vb = LBv[b * CAP : (b + 1) * CAP]
            full = Vb[:6144].reshape(6, 8, P, R).transpose(1, 3, 0, 2).reshape(64, 6 * P)
            lbt[:, (NSUB * b) * P : (NSUB * b + 6) * P] = full
            tail = Vb[6144:].reshape(4, P, R).transpose(0, 2, 1).reshape(32, P)
            lbt[:32, (NSUB * b + 6) * P : (NSUB * b + 7) * P] = tail

        in_maps.append({"tab": tab, "idx": idx16, "lbt": lbt, "aeffb": aeffb})
        host_info.append((slot_src, valid, overflow))
    return in_maps, host_info, (E, LB, a_scaled)


def collect(results, host_info, tabs, x):
    """Un-permute the banked bf16 output; host-patches (never-in-practice) overflow."""
    E, LB, a_scaled = tabs
    xi = np.asarray(x).reshape(-1).astype(np.int64)
    res = np.empty((NTOK, D), dtype=np.float32)
    pm_full = np.tile(_PMAP, NBANK)
    cm_full = (np.repeat(np.arange(NBANK) * CCOL, CAP) + np.tile(_CMAP, NBANK))
    for c in range(NCORES):
        slot_src, valid, overflow = host_info[c]
        oc = np.asarray(results[c]["out"])  # [P, NCOL, D] bf16
        v = valid.reshape(-1)
        res[slot_src.reshape(-1)[v]] = oc[pm_full[v], cm_full[v], :].astype(np.float32)
        for b, toks in overflow.items():
            ids = xi[toks]
            res[toks] = E[ids] + LB[ids] @ a_scaled
    return res.reshape(B, L, D)


def kernel(x, embedding_weight, lora_A, lora_B, rank_pattern):
    from concourse.bass_utils import run_bass_kernel_spmd

    x = np.asarray(x)
    in_maps, host_info, tabs = prepare_in_maps(
        x, embedding_weight, lora_A, lora_B, rank_pattern
    )
    nc = _get_nc()
    res = run_bass_kernel_spmd(nc, in_maps, list(range(NCORES))).results
    return collect(res, host_info, tabs, x)
